# revision 1
# baseline (speedup 1.0000x reference)
"""Trainium2 Bass kernel for nn_Attention_40570261078258.

Computes, for x:(8,128,64,64), Wq/Wk/Wv:(128,128), bq/bk/bv:(128,):
    xf = x.reshape(N, C, L);  L = 4096
    q/k/v = W @ xf + b                  -> (N, L, C) logical
    scores = q @ k^T / sqrt(C)          -> (N, L, L)
    attn = softmax(scores, axis=0)      # over the BATCH axis (torch legacy dim=0)
    out = attn @ v                      -> (N, L, C)
    return x + out.reshape(N, C, H, W)  # reinterpreting (L,C) memory as (C,H,W)

Sharding: the softmax couples all batch elements at each (l, m) pair, so
batch-parallel would need a 64MB denominator all-reduce. Instead we shard the
query dim L across the 8 cores: each core handles l in [d*512, (d+1)*512) for
ALL batch elements, making the softmax entirely local (no collectives).
Each core redundantly computes k/v for all of L (cheap vs. attention).

Engine assignment (hardware-legal: gpsimd cannot touch PSUM or run
TensorScalarPtr, and the DVE has no divide op):
  PE   : projections (f32r) + scores (bf16) + attn@v (bf16)     ~152us
  ACT  : the 16.7M-element exp + k/q/epilogue PSUM evictions    ~176us
  DVE  : softmax tree + reciprocal + half the normalize muls,
         v-hat eviction (needs a free-dim bias)                 ~177us
  POOL : the other half of the normalize muls (SBUF-only TT)    ~143us
Attention jobs run per 128-query x 128-key tile through a 3-slot PSUM
rotation (2-bank tiles; the AV accumulator takes the last 2 banks), with
projections interleaved into the job stream so the phases overlap; the
softmax elementwise ops process two jobs at a time (paired E tiles) to
amortize per-op fixed costs.

SPMD: all cores run the identical graph; the per-core slice is selected by the
host passing a per-core q-input slice (xq). The device returns the attention
output in (c,l)-major tiles; the host reinterleaves and adds the residual.
"""

import math

import numpy as np

import concourse.bacc as bacc
import concourse.bass as bass
import concourse.mybir as mybir
import concourse.tile as tile
from concourse.bass_utils import run_bass_kernel_spmd

N, C, H, W = 8, 128, 64, 64
L = H * W            # 4096 pixels
NCORES = 8
LSH = L // NCORES    # 512 query positions per core
NLH = 4              # l-quarters per core
LHW = LSH // NLH     # 128 l per quarter
NMT = L // 128       # 32 key/value tiles of 128

FP = mybir.dt.float32
FR = mybir.dt.float32r
BF = mybir.dt.bfloat16
AF = mybir.ActivationFunctionType
ALU = mybir.AluOpType

SKEW = 8           # jobs of lookahead between scores and softmax/AV
E_BUFS = SKEW // 2 + 3  # E pair-tile slots (4KB/partition each)
A_BUFS = 3         # attn pair-tile slots
MUL_GD = 5         # normalize-mul batch groups on DVE (rest on gpsimd)

# Set by test harness to capture a profile.
TRACE = False
LAST_RESULTS = None


def build():
    nc = bacc.Bacc(
        "TRN2",
        target_bir_lowering=False,
        debug=False,
        enable_asserts=True,
        num_devices=NCORES,
    )

    # x and the transposed weights are declared float32r (same bits as f32)
    # so the projection matmuls run at full PE rate without a bf16 pre-cast.
    xk = nc.dram_tensor("xk", [N, C, L], FR, kind="ExternalInput").ap()
    xq = nc.dram_tensor("xq", [N, C, LSH], FR, kind="ExternalInput").ap()
    # Weights arrive pre-transposed and packed [WqT | WkT | WvT] so the
    # whole set loads in one DMA (HWDGE descriptor passes serialize at
    # ~625ns each); bq/bk pack likewise.
    wall = nc.dram_tensor("wall", [C, 3 * C], FR, kind="ExternalInput").ap()
    bqk = nc.dram_tensor("bqk", [C, 2], FP, kind="ExternalInput").ap()
    bv = nc.dram_tensor("bv", [1, C], FP, kind="ExternalInput").ap()
    # Attention output in (c, l)-major layout, bf16 (the host upcasts during
    # the (l,c) reinterleave + residual add; out values are O(20) so bf16
    # rounding adds ~2e-3 relative error, well inside the 2e-2 gate).
    out = nc.dram_tensor("out", [N, NLH, C, LHW], BF, kind="ExternalOutput").ap()

    with tile.TileContext(nc) as tc:
        _emit(nc, tc, xk, xq, wall, bqk, bv, out)

    nc.compile()
    return nc


def _emit(nc, tc, xk, xq, wall, bqk, bv, out):
    from contextlib import ExitStack

    with ExitStack() as ctx:
        cpool = ctx.enter_context(tc.tile_pool(name="const", bufs=1))
        resid = ctx.enter_context(tc.tile_pool(name="resident", bufs=1))

        # --- constants (3 DMAs total; HWDGE passes are ~625ns each) --------
        wallt = cpool.tile([C, 3 * C], FR, tag="wall")
        nc.sync.dma_start(wallt[:], wall)
        bqk_t = cpool.tile([C, 2], FP, tag="bqk")
        nc.sync.dma_start(bqk_t[:], bqk)
        bv_f = cpool.tile([1, C], FP, tag="bvf")
        nc.sync.dma_start(bv_f[:], bv)
        bq_t = bqk_t[:, 0:1]
        bk_t = bqk_t[:, 1:2]
        ones_row = cpool.tile([1, C], FP, tag="ones")
        nc.vector.memset(ones_row[:], 1.0)
        # bv replicated across partitions (rank-1 ones @ bv matmul)
        bv_rep = cpool.tile([128, C], FP, tag="bvrep")

        with tc.tile_pool(name="wpsum", bufs=1, space="PSUM") as wpsum_pool:
            pb = wpsum_pool.tile([128, C], FP, tag="wps")
            nc.tensor.matmul(pb[:], ones_row[:], bv_f[:], start=True, stop=True)
            nc.vector.tensor_copy(bv_rep[:], pb[:])
        # WvT padded to 256 columns of zeros so the float32r vT matmuls hit
        # the >=256 free-dim full-rate path (junk half never read).
        wvpad = cpool.tile([C, 2 * C], FR, tag="wvpad")
        nc.vector.memset(wvpad[:].bitcast(FP), 0.0)
        nc.vector.tensor_copy(wvpad[:, 0:C], wallt[:, 2 * C : 3 * C])

        # --- resident activations -----------------------------------------
        # q_sb[n]: (c, l) for this core's l-slice;  k_sb[n]: (c, m) full L;
        # vT_sb[n]: (m % 128, 32*128) i.e. 32 chunks of (m,c), all bf16.
        q_sb = [
            resid.tile([C, LSH], BF, tag=f"q{n}", name=f"q_sb{n}") for n in range(N)
        ]
        k_sb = [
            resid.tile([C, L], BF, tag=f"k{n}", name=f"k_sb{n}") for n in range(N)
        ]
        vT_sb = [
            resid.tile([128, NMT * C], BF, tag=f"v{n}", name=f"vT_sb{n}")
            for n in range(N)
        ]

        # --- fused projection + attention stream ---------------------------
        # Projections run chunk-outer (all batches per m-chunk) so attention
        # jobs for m-tile range [8b, 8b+8) unblock as soon as chunk b lands;
        # the emission interleaves them so PE never drains between phases.
        # One rotating 2-slot PSUM pool serves q/k/v projections AND scores
        # (4 banks), the AV accumulators take the other 4 banks.
        inv_sqrt_c = 1.0 / math.sqrt(C)
        wqT_r = wallt[:, 0:C]
        wkT_r = wallt[:, C : 2 * C]
        wvpad_r = wvpad[:]
        with (
            tc.tile_pool(name="xin", bufs=3) as xin_pool,
            tc.tile_pool(name="wrk", bufs=3, space="PSUM") as wrk_psum,
            tc.tile_pool(name="avp", bufs=1, space="PSUM") as av_psum,
            tc.tile_pool(name="soft", bufs=1) as soft_pool,
            tc.tile_pool(name="ost", bufs=1) as ost_pool,
        ):
            def emit_q_proj(n):
                xt = xin_pool.tile([C, LSH], FR, tag="xq", bufs=2)
                nc.sync.dma_start(xt[:], xq[n])
                pq = wrk_psum.tile([128, 1024], FP, tag="ps")
                nc.tensor.matmul(
                    pq[:, 0:512], wqT_r, xt[:], start=True, stop=True
                )
                nc.scalar.activation(
                    q_sb[n][:], pq[:, 0:512], AF.Identity, bias=bq_t
                )

            def emit_kv_chunk(n, bch, xt, k_act):
                # One 1024-wide m-chunk of k-hat and v-hat for batch n.
                xr_ = xt[:]
                # Both 512-wide k matmuls land in one 2-bank PSUM tile so
                # the bias-adding eviction is one 1024-wide op. That
                # eviction runs on ACT (Identity + per-partition bias),
                # which is otherwise idle while projections flow.
                pk = wrk_psum.tile([128, 1024], FP, tag="ps")
                for half in range(2):
                    nc.tensor.matmul(
                        pk[:, half * 512 : (half + 1) * 512],
                        wkT_r,
                        xr_[:, half * 512 : (half + 1) * 512],
                        start=True,
                        stop=True,
                    )
                if k_act:
                    nc.scalar.activation(
                        k_sb[n][:, bch * 1024 : (bch + 1) * 1024],
                        pk[:],
                        AF.Identity,
                        bias=bk_t,
                    )
                else:
                    nc.vector.tensor_scalar_add(
                        k_sb[n][:, bch * 1024 : (bch + 1) * 1024],
                        pk[:],
                        bk_t,
                    )
                for half in range(2):
                    ch = 2 * bch + half
                    # vT chunks: out[m,c] = sum_c' x[c',m] WvT[c',c] + bv[c].
                    # Each 128-m sub-tile occupies a 256-wide PSUM slice
                    # (f32r full-rate needs >=256 free; upper half junk).
                    pv = wrk_psum.tile([128, 1024], FP, tag="ps")
                    for sub in range(4):
                        sl = slice(sub * 256, sub * 256 + 256)
                        nc.tensor.matmul(
                            pv[:, sl],
                            xr_[:, half * 512 + sub * 128 :
                                half * 512 + (sub + 1) * 128],
                            wvpad_r,
                            start=(sub % 2 == 0),
                            stop=(sub % 2 == 1),
                        )
                    # Strided 3D read picks the real 128 of each 256; bv
                    # rides along via the broadcast add.
                    nc.vector.scalar_tensor_tensor(
                        vT_sb[n][:, ch * 512 : (ch + 1) * 512].rearrange(
                            "p (s c) -> p s c", s=4
                        ),
                        pv[:].rearrange("p (s c2) -> p s c2", s=4)[:, :, 0:128],
                        1.0,
                        bv_rep[:].unsqueeze(1).broadcast_to((128, 4, C)),
                        ALU.mult,
                        ALU.add,
                    )

            def emit_epilogue(avp_prev, lh_prev):
                # Evict the PSUM accumulator (freeing the av slot for the
                # next l-quarter) to SBUF staging on gpsimd, then DMA out.
                ob = ost_pool.tile([128, 1024], BF, tag="ob", bufs=2)
                nc.scalar.activation(ob[:], avp_prev[:], AF.Copy)
                # One batched DMA for all 8 batches (8 separate dma_starts
                # serialize ~625ns each on the HWDGE descriptor generator).
                nc.sync.dma_start(
                    out[:, lh_prev].rearrange("n c l -> c n l"),
                    ob[:].rearrange("p (n l) -> p n l", n=N),
                )

            pend = {}   # (lh, even mt) -> E pair tile (128, 2 x 8n x 128l)
            avps = {}   # lh -> accumulator tile

            def emit_scores(lh, mt):
                # E tiles are allocated per PAIR of consecutive m-tiles so
                # the downstream softmax elementwise ops run 2048 wide (the
                # per-op fixed costs amortize) while the PSUM stays at
                # 2-bank granularity for the 3-slot rotation.
                l0 = lh * LHW
                if mt % 2 == 0:
                    e_new = soft_pool.tile(
                        [128, 2048], BF, tag="E", bufs=E_BUFS, name="e_pair"
                    )
                    pend[(lh, mt)] = e_new
                e = pend[(lh, mt - mt % 2)]
                eh = e[:, (mt % 2) * 1024 : (mt % 2) * 1024 + 1024]
                ps = wrk_psum.tile([128, 1024], FP, tag="ps")
                for n in range(N):
                    nc.tensor.matmul(
                        ps[:, n * LHW : (n + 1) * LHW],
                        k_sb[n][:, mt * 128 : (mt + 1) * 128],
                        q_sb[n][:, l0 : l0 + LHW],
                        start=True,
                        stop=True,
                    )
                nc.scalar.activation(eh, ps[:], AF.Exp, scale=inv_sqrt_c)

            njob = [0]

            def emit_soft_av(lh, mt0, split_divide=False):
                # Softmax + AV for the job pair (lh, mt0), (lh, mt0+1).
                if mt0 == 0:
                    # One (c, 8n x 128l) accumulator, 2 PSUM banks; group
                    # start/stop is per 2KB bank (4 batch slices each).
                    avps[lh] = av_psum.tile(
                        [128, 1024], FP, tag="av", name=f"avp{lh}"
                    )
                avp = avps[lh]
                e = pend.pop((lh, mt0))
                e3 = e[:].rearrange("p (j h) -> p j h", j=2)
                # Batch-sum tree (TensorTensor, 2x packed bf16 on DVE; the
                # wide level rotates onto gpsimd every few pairs), then a
                # broadcast DIVIDE normalizes all 16 batch groups — no
                # separate reciprocal.
                s1 = soft_pool.tile([128, 1024], BF, tag="zt1", bufs=3)
                s13 = s1[:].rearrange("p (j h) -> p j h", j=2)
                nc.vector.tensor_tensor(
                    s13, e3[:, :, 0:512], e3[:, :, 512:1024], ALU.add
                )
                s2 = soft_pool.tile([128, 512], BF, tag="zt2", bufs=2)
                s23 = s2[:].rearrange("p (j h) -> p j h", j=2)
                nc.vector.tensor_tensor(
                    s23, s13[:, :, 0:256], s13[:, :, 256:512], ALU.add
                )
                zr = soft_pool.tile([128, 2 * LHW], BF, tag="zr", bufs=3)
                nc.vector.tensor_tensor(
                    zr[:].rearrange("p (j l) -> p j l", j=2),
                    s23[:, :, 0:LHW],
                    s23[:, :, LHW : 2 * LHW],
                    ALU.add,
                )
                r = soft_pool.tile([128, 2 * LHW], BF, tag="r", bufs=4)
                with nc.allow_low_precision(
                    "softmax denom is a sum of 16 O(1..500) exps; bf16 ok"
                ):
                    nc.vector.reciprocal(r[:], zr[:])
                a = soft_pool.tile([128, 2048], BF, tag="A", bufs=2)
                # Normalize attn = E * (1/Z): per-job 3D broadcast
                # multiplies, batch groups split DVE / gpsimd.
                with nc.allow_low_precision(
                    "softmax normalize; bf16 attn weights are plenty"
                ):
                    for j in range(2):
                        gd = 8 if (lh, mt0) == (NLH - 1, NMT - 2) else MUL_GD
                        ej = e[:, j * 1024 : (j + 1) * 1024].rearrange(
                            "p (g l) -> p g l", g=8
                        )
                        aj = a[:, j * 1024 : (j + 1) * 1024].rearrange(
                            "p (g l) -> p g l", g=8
                        )
                        rj = r[:, j * LHW : (j + 1) * LHW]
                        nc.vector.tensor_mul(
                            aj[:, :gd],
                            ej[:, :gd],
                            rj.unsqueeze(1).broadcast_to((128, gd, LHW)),
                        )
                        if gd < 8:
                            nc.gpsimd.tensor_mul(
                                aj[:, gd:],
                                ej[:, gd:],
                                rj.unsqueeze(1).broadcast_to(
                                    (128, 8 - gd, LHW)
                                ),
                            )
                for j in range(2):
                    mt = mt0 + j
                    for n in range(N):
                        nc.tensor.matmul(
                            avp[:, n * LHW : (n + 1) * LHW],
                            vT_sb[n][:, mt * C : (mt + 1) * C],
                            a[:, j * 1024 + n * LHW : j * 1024 + (n + 1) * LHW],
                            start=(mt == 0 and n % 4 == 0),
                            stop=(mt == NMT - 1 and n % 4 == 3),
                        )
                if mt0 == NMT - 2:
                    emit_epilogue(avps.pop(lh), lh)

            # Emission stream: q projections, then k/v chunks interleaved
            # with the attention jobs they unblock. Chunk b (m in
            # [1024b, 1024b+1024)) enables jobs (lh=0, mt in [8b, 8b+8));
            # lh=1 jobs run after all projections. The SKEW-deep pend queue
            # software-pipelines scores against softmax/AV throughout.
            def proj_n(bch, n, k_act):
                xt = xin_pool.tile([C, 1024], FR, tag="x", bufs=3)
                nc.sync.dma_start(
                    xt[:], xk[n, :, bch * 1024 : (bch + 1) * 1024]
                )
                emit_kv_chunk(n, bch, xt, k_act)

            pairs = []

            def push_job(job, split_divide=False, skew=None):
                emit_scores(*job)
                lh, mt = job
                if mt % 2 == 1:
                    pairs.append(((lh, mt - 1), split_divide))
                while 2 * len(pairs) > (SKEW if skew is None else skew):
                    j, s = pairs.pop(0)
                    emit_soft_av(*j, split_divide=s)

            # bch 0 is the warmup (jobs need all 8 batches of a k/v tile):
            # no attention work exists yet, so its evictions spread across
            # DVE and POOL. bch 1..3 then interleave one batch-projection
            # per attention job (POOL takes the evictions, DVE half the v),
            # and the projection-free tail hands POOL part of the divides.
            for n in range(N):
                emit_q_proj(n)
            for n in range(N):
                proj_n(0, n, True)
            for bch in range(1, 4):
                for n in range(N):
                    push_job((0, 8 * (bch - 1) + n))
                    proj_n(bch, n, True)
            for mt in range(24, 32):
                push_job((0, mt), split_divide=True)
            for lh in range(1, NLH):
                for mt in range(32):
                    tail = lh == NLH - 1 and mt >= 28
                    push_job((lh, mt), split_divide=True,
                             skew=4 if tail else None)
            for jb, s in pairs:
                emit_soft_av(*jb, split_divide=s)


_NC = None


def _get_nc():
    global _NC
    if _NC is None:
        _NC = build()
    return _NC


def kernel(x, Wq, bq, Wk, bk, Wv, bv):
    global LAST_RESULTS
    x = np.ascontiguousarray(np.asarray(x, dtype=np.float32))
    wall = np.ascontiguousarray(
        np.concatenate(
            [np.asarray(w, dtype=np.float32).T for w in (Wq, Wk, Wv)], axis=1
        )
    )
    bqk = np.ascontiguousarray(
        np.stack(
            [
                np.asarray(bq, dtype=np.float32).ravel(),
                np.asarray(bk, dtype=np.float32).ravel(),
            ],
            axis=1,
        )
    )
    bv = np.asarray(bv, dtype=np.float32).reshape(1, C)

    xf = x.reshape(N, C, L)
    xflat = x.reshape(N, C * H * W)

    in_maps = []
    for d in range(NCORES):
        lo = d * LSH
        in_maps.append(
            {
                "xk": xf,
                "xq": np.ascontiguousarray(xf[:, :, lo : lo + LSH]),
                "wall": wall,
                "bqk": bqk,
                "bv": bv,
            }
        )

    nc = _get_nc()
    res = run_bass_kernel_spmd(
        nc, in_maps, core_ids=list(range(NCORES)), trace=TRACE
    )
    LAST_RESULTS = res
    # Device returns attention output in (c, l)-major tiles; reinterleave to
    # the reference's flat (l, c) order and add the residual here.
    att = np.concatenate(
        [
            res.results[d]["out"]
            .astype(np.float32)
            .transpose(0, 1, 3, 2)
            .reshape(N, LSH * C)
            for d in range(NCORES)
        ],
        axis=1,
    )
    return (xflat + att).reshape(N, C, H, W)

